# revision 56
# baseline (speedup 1.0000x reference)
"""Trainium2 Bass kernel for a pre-norm transformer decoder layer (fp8).

Sharding: 8 cores = 4 batches x 2 sequence-stripe halves.  Core c handles
batch b=c//2 and the 1024 queries q with (q mod 512)//256 == c%2.

Token order on-chip is per-core PERMUTED, own-phase-first:
tau = (ph==own)? 0:1)*1024 + j*256 + off  for global token j*512 + ph*256 + off.
Own tokens therefore always occupy tau in [0,1024) -> the program is
identical across cores; only input data (x, mask, residual slab) differs.

All heavy GEMMs run in fp8e4 (weights pre-scaled x64 / x128 on host) with
DoubleRow perf mode: 256-deep contraction per matmul, 2x PE throughput.
Activations are stored fp8e4.  On-chip dataflow is feature-major
([D, tokens]); matmuls contract over the partition dim with zero on-chip
transposes.  LayerNorm stats use fp8 ones-DoubleRow matmuls; mean/rstd rows
broadcast across partitions via K=1 f32r matmuls.  Causal softmax skips
max-subtraction (scores are O(1)); exp is scaled by 1/sqrt(D); a 0/1 bf16
mask multiplies the 8 diagonal-zone key blocks.  LN2 is batched after the
attention loop so the ACT sqrt/exp table sets load once each.  PSUM
evacuations run on DVE (cheaper than ACT at N=512); fc1's relu+bias+scale
stays on ACT.  y1 (= x + attn_out) stays resident in SBUF through the FFN.
"""

import numpy as np
import ml_dtypes

import concourse.bass as bass
import concourse.tile as tile
import concourse.mybir as mybir
from concourse.bass_utils import run_bass_kernel_spmd

FP32 = mybir.dt.float32
F32R = mybir.dt.float32r
BF16 = mybir.dt.bfloat16
F8 = mybir.dt.float8e4
DRM = mybir.MatmulPerfMode.DoubleRow
AOP = mybir.AluOpType
ACT = mybir.ActivationFunctionType
EPS = 1e-5
WS = 64.0     # host pre-scale on W_q/W_k/W_v/W_o/fc1_w
WS2 = 128.0   # host pre-scale on fc2_w


def _split_drain_waits(nc, max_waits=1):
    """walrus here rejects >max_waits sync waits per instruction; split
    extras onto preceding single-wait NoOps on the same engine."""
    for f in nc.m.functions:
        for bb in f.blocks:
            insts = list(bb.instructions)
            out, changed = [], False
            for inst in insts:
                si = inst.sync_info
                if si is not None and len(si.on_wait) > max_waits:
                    waits = list(si.on_wait)
                    for j, w in enumerate(waits[:-max_waits]):
                        out.append(mybir.InstNoOp(
                            name=f"{inst.name}_sw{j}", ins=[], outs=[],
                            engine=inst.engine,
                            sync_info=mybir.SyncInfo(on_wait=[w],
                                                     on_update=[])))
                    inst.sync_info = mybir.SyncInfo(
                        on_wait=waits[-max_waits:],
                        on_update=list(si.on_update))
                    changed = True
                out.append(inst)
            if changed:
                bb.instructions = out


def build_decoder_nc(S=2048, D=1024, F=4096, apply_ln_affine=False,
                     debug=False, surgery=True, repeat=1):
    """Single-core Bass program (per-core shapes, fp8 DoubleRow)."""
    DC = D // 128          # 8 feature blocks
    DCP = DC // 2          # 4 DoubleRow feature pairs
    FC = F // 128          # 32
    FCP = FC // 2          # 16
    OWN = S // 2           # 1024 own tokens
    NT = S // 512          # 4 full-S token tiles
    NTO = OWN // 512       # 2 own token tiles
    NKT = S // 128         # 16 token blocks
    scale_q = 1.0 / float(np.sqrt(D))

    nc = bass.Bass()

    xbf = nc.dram_tensor("xbf", [D, S], BF16, kind="ExternalInput")
    xow = nc.dram_tensor("xow", [D, OWN], FP32, kind="ExternalInput")
    mask8 = nc.dram_tensor("mask8", [1024, 512], BF16, kind="ExternalInput")
    wq8 = nc.dram_tensor("wq8", [D, D], F8, kind="ExternalInput")
    wk8 = nc.dram_tensor("wk8", [D, D], F8, kind="ExternalInput")
    wv8 = nc.dram_tensor("wv8", [D, D], F8, kind="ExternalInput")
    wob = nc.dram_tensor("wob", [D, D], BF16, kind="ExternalInput")
    f18 = nc.dram_tensor("f18", [D, F], F8, kind="ExternalInput")
    f28 = nc.dram_tensor("f28", [F, D], F8, kind="ExternalInput")
    fc1b = nc.dram_tensor("fc1b", [F], FP32, kind="ExternalInput")
    fc2b = nc.dram_tensor("fc2b", [D], FP32, kind="ExternalInput")
    lnp = None
    if apply_ln_affine:
        lnp = nc.dram_tensor("lnp", [4, D], FP32, kind="ExternalInput")
    outT = nc.dram_tensor("outT", [D, OWN], FP32, kind="ExternalOutput")

    x_r = xbf.rearrange("(c p) s -> p c s", p=128)
    xow_r = xow.rearrange("(c p) s -> p c s", p=128)
    mask_r = mask8.rearrange("(j p) q -> p j q", p=128)
    wq_r = wq8.rearrange("(c p) e -> p c e", p=128)
    wk_r = wk8.rearrange("(c p) e -> p c e", p=128)
    wv_r = wv8.rearrange("(c p) e -> p c e", p=128)
    wo_r = wob.rearrange("(c p) e -> p c e", p=128)
    f18_r = f18.rearrange("(c p) f -> p c f", p=128)
    f28_r = f28.rearrange("(c p) d -> p c d", p=128)
    fc1b_r = fc1b.rearrange("(c p) -> p c", p=128)
    fc2b_r = fc2b.rearrange("(c p) -> p c", p=128)
    outT_r = outT.rearrange("(c p) s -> p c s", p=128)

    with tile.TileContext(nc) as tc:
        with (
            tc.tile_pool(name="consts", bufs=1) as consts,
            tc.tile_pool(name="work", bufs=3) as work,
        ):
            ones2w = consts.tile([128, 2, 16], F8, tag="ones2")
            nc.vector.memset(ones2w, 1.0)
            ones2 = ones2w[:, :, 0:1]   # pair-dim step 16B (ISA DR rule)
            ones_col = consts.tile([128, 1], BF16, tag="ones_col")
            nc.vector.memset(ones_col, 1.0)
            ones_row = consts.tile([1, 128], F32R, tag="ones_row")
            ones_row_f = consts.tile([1, 128], FP32, tag="ones_row_f")
            nc.vector.memset(ones_row_f, 1.0)
            nc.vector.tensor_copy(ones_row, ones_row_f)
            eps_t = consts.tile([1, 1], FP32, tag="eps")
            nc.vector.memset(eps_t, EPS)
            fc1b_t = consts.tile([128, FC], FP32, tag="fc1b")
            nc.sync.dma_start(fc1b_t, fc1b_r)
            fc2b_t = consts.tile([128, DC], FP32, tag="fc2b")
            nc.sync.dma_start(fc2b_t, fc2b_r)
            maskt = consts.tile([128, 8, 512], BF16, tag="mask")
            nc.sync.dma_start(maskt, mask_r)
            lnp_t = None
            if apply_ln_affine:
                lnp_t = consts.tile([128, 4, DC], FP32, tag="lnp")
                nc.sync.dma_start(
                    lnp_t, lnp.rearrange("g (c p) -> p g c", p=128))

            def ln_rows(ps_sx, ps_sq, n):
                """PSUM stat rows -> (mu, rstd) f32r work rows of width n."""
                mu = work.tile([1, n], F32R, tag="r_mu", bufs=2)
                ms = work.tile([1, n], F32R, tag="r_ms", bufs=2)
                msq = work.tile([1, n], F32R, tag="r_msq", bufs=2)
                nc.vector.tensor_scalar_mul(mu, ps_sx, 1.0 / D)
                nc.vector.tensor_scalar_mul(ms, ps_sq, 1.0 / D)
                nc.vector.tensor_mul(msq, mu, mu)
                nc.vector.tensor_sub(ms, ms, msq)
                nc.scalar.activation(ms, ms, ACT.Sqrt, bias=eps_t)
                with nc.allow_low_precision(reason="rstd row f32r"):
                    nc.vector.reciprocal(ms, ms)
                return mu, ms

            def bcast(row, n, ps_pool, tag="bc"):
                """Broadcast a [1,n] f32r row across 128 partitions -> SBUF.
                Returns a [128, 1, n] tile (unit dim for broadcast_to)."""
                ps = ps_pool.tile([128, n], FP32, tag=tag)
                nc.tensor.matmul(ps, ones_row, row)
                sb = work.tile([128, 1, n], FP32, tag=tag + "_sb", bufs=2)
                nc.vector.tensor_copy(sb, ps)
                return sb

            def ln_stats(src3, sl, ln_ps, sqp, src_f8):
                """Ones-matmul stats for token tile src3[:, :, sl].
                x-sums: fp8 DR if src is fp8, else bf16 non-DR.
                sq-sums: always fp8 DR (squares stored fp8)."""
                n = sl.stop - sl.start
                sq = sqp.tile([128, DC, n], F8, tag="sq")
                for i in range(DCP):
                    nc.gpsimd.tensor_mul(sq[:, 2 * i:2 * i + 2, :],
                                         src3[:, 2 * i:2 * i + 2, sl],
                                         src3[:, 2 * i:2 * i + 2, sl])
                ps_sx = ln_ps.tile([1, n], FP32, tag="ps_sx")
                ps_sq = ln_ps.tile([1, n], FP32, tag="ps_sq")
                if src_f8:
                    for i in range(DCP):
                        nc.tensor.matmul(ps_sx, ones2,
                                         src3[:, 2 * i:2 * i + 2, sl],
                                         start=(i == 0),
                                         stop=(i == DCP - 1),
                                         perf_mode=DRM)
                else:
                    for dc in range(DC):
                        nc.tensor.matmul(ps_sx, ones_col, src3[:, dc, sl],
                                         start=(dc == 0),
                                         stop=(dc == DC - 1))
                for i in range(DCP):
                    nc.tensor.matmul(ps_sq, ones2, sq[:, 2 * i:2 * i + 2, :],
                                     start=(i == 0), stop=(i == DCP - 1),
                                     perf_mode=DRM)
                return ps_sx, ps_sq

            def ln_apply(src3, ssl, dst3, dsl, mb_sb, rb_sb, g_idx, b_idx):
                n = dsl.stop - dsl.start
                if not apply_ln_affine:
                    # one 3D op per stage; rows broadcast over the dc dim
                    mb_b = mb_sb.broadcast_to([128, DC, n])
                    rb_b = rb_sb.broadcast_to([128, DC, n])
                    t1 = work.tile([128, DC, n], BF16, tag="lnt3", bufs=2)
                    nc.gpsimd.tensor_sub(t1, src3[:, :, ssl], mb_b)
                    nc.vector.tensor_mul(dst3[:, :, dsl], t1, rb_b)
                    return
                for dc in range(DC):
                    t1 = work.tile([128, n], BF16, tag="lnt")
                    nc.vector.tensor_sub(t1, src3[:, dc, ssl],
                                         mb_sb[:, 0, :])
                    nc.vector.scalar_tensor_tensor(
                        dst3[:, dc, dsl], t1,
                        lnp_t[:, g_idx, dc:dc + 1], rb_sb[:, 0, :],
                        AOP.mult, AOP.mult)
                    nc.vector.tensor_scalar_add(
                        dst3[:, dc, dsl], dst3[:, dc, dsl],
                        lnp_t[:, b_idx, dc:dc + 1])

            for _rep in range(repeat):
              with tc.tile_pool(name="outer", bufs=1) as outer:
                y1 = outer.tile([128, DC, OWN], FP32, tag="y1")
                xn2 = outer.tile([128, DC, OWN], F8, tag="xn2")
                with tc.tile_pool(name="L1", bufs=1) as L1:
                    KT = L1.tile([128, DC, S], F8, tag="KT")
                    VT = L1.tile([128, NKT, D], F8, tag="VT")
                    QT = L1.tile([128, DC, OWN], F8, tag="QT")

                    # ---- phase A: LN1 + K/V/Q projections ----
                    with (
                        tc.tile_pool(name="xsp", bufs=1) as xsp,
                        tc.tile_pool(name="xtp", bufs=2) as xtp,
                        tc.tile_pool(name="sqp", bufs=2) as sqp,
                        tc.tile_pool(name="wp", bufs=2) as wp,
                        tc.tile_pool(name="lnps", bufs=1,
                                     space="PSUM") as lnps,
                        tc.tile_pool(name="bcps", bufs=1,
                                     space="PSUM") as bcps,
                        tc.tile_pool(name="pps", bufs=5, space="PSUM") as pps,
                    ):
                        xn = xsp.tile([128, DC, S], F8, tag="xn")
                        WK = wp.tile([128, DC, D], F8, tag="w")
                        for dc in range(DC):
                            nc.sync.dma_start(WK[:, dc, :], wk_r[:, dc, :])
                        WQ = wp.tile([128, DC, D], F8, tag="w")
                        for dc in range(DC):
                            nc.sync.dma_start(WQ[:, dc, :], wq_r[:, dc, :])

                        def ln1_tile(ti):
                            sl = slice(ti * 512, (ti + 1) * 512)
                            xt = xtp.tile([128, DC, 512], BF16, tag="xt")
                            for dc in range(DC):
                                nc.sync.dma_start(xt[:, dc, :],
                                                  x_r[:, dc, sl])
                            fsl = slice(0, 512)
                            ps_sx, ps_sq = ln_stats(xt, fsl, lnps, sqp,
                                                    False)
                            mu, rs = ln_rows(ps_sx, ps_sq, 512)
                            mb_sb = bcast(mu, 512, bcps)
                            rb_sb = bcast(rs, 512, bcps)
                            ln_apply(xt, fsl, xn, sl, mb_sb, rb_sb, 0, 1)

                        def kproj_tile(ti):
                            sl = slice(ti * 512, (ti + 1) * 512)
                            for ec in range(DC):
                                ps = pps.tile([128, 512], FP32, tag="pps")
                                for i in range(DCP):
                                    nc.tensor.matmul(
                                        ps,
                                        WK[:, 2 * i:2 * i + 2,
                                           ec * 128:(ec + 1) * 128],
                                        xn[:, 2 * i:2 * i + 2, sl],
                                        start=(i == 0), stop=(i == DCP - 1),
                                        perf_mode=DRM)
                                nc.scalar.mul(KT[:, ec, sl], ps, 1.0 / WS)

                        ln1_tile(0)
                        ln1_tile(1)
                        for ti in range(NT):
                            kproj_tile(ti)
                            if ti + 2 < NT:
                                ln1_tile(ti + 2)
                        # Q projection next (own slab = tau in [0, OWN)),
                        # so attention scores can start before V finishes
                        for tj in range(NTO):
                            sl = slice(tj * 512, (tj + 1) * 512)
                            for ec in range(DC):
                                ps = pps.tile([128, 512], FP32, tag="pps")
                                for i in range(DCP):
                                    nc.tensor.matmul(
                                        ps,
                                        WQ[:, 2 * i:2 * i + 2,
                                           ec * 128:(ec + 1) * 128],
                                        xn[:, 2 * i:2 * i + 2, sl],
                                        start=(i == 0), stop=(i == DCP - 1),
                                        perf_mode=DRM)
                                nc.scalar.mul(QT[:, ec, sl], ps, 1.0 / WS)
                        # V projection (token blocks stationary)
                        WV = wp.tile([128, DC, D], F8, tag="w")
                        for dc in range(DC):
                            nc.sync.dma_start(WV[:, dc, :], wv_r[:, dc, :])
                        for tb in range(NKT):
                            for eh in range(2):
                                esl = slice(eh * 512, (eh + 1) * 512)
                                ps = pps.tile([128, 512], FP32, tag="pps")
                                for i in range(DCP):
                                    nc.tensor.matmul(
                                        ps,
                                        xn[:, 2 * i:2 * i + 2,
                                           tb * 128:(tb + 1) * 128],
                                        WV[:, 2 * i:2 * i + 2, esl],
                                        start=(i == 0), stop=(i == DCP - 1),
                                        perf_mode=DRM)
                                nc.scalar.mul(VT[:, tb, esl], ps, 1.0 / WS)

                    # ---- attention ----
                    with (
                        tc.tile_pool(name="wop", bufs=1) as wop,
                        tc.tile_pool(name="xop", bufs=1) as xop,
                        tc.tile_pool(name="ptp", bufs=12) as ptp,
                        tc.tile_pool(name="attw", bufs=2) as attw,
                        tc.tile_pool(name="sps", bufs=3, space="PSUM") as sps,
                        tc.tile_pool(name="dnp", bufs=1, space="PSUM") as dnp,
                        tc.tile_pool(name="avp", bufs=2, space="PSUM") as avp,
                        tc.tile_pool(name="bc1", bufs=1, space="PSUM") as bc1,
                    ):
                        WO = wop.tile([128, DC, D], BF16, tag="wo")
                        for dc in range(DC):
                            nc.sync.dma_start(WO[:, dc, :], wo_r[:, dc, :])

                        for jq in range(NTO):
                            qsl = slice(jq * 512, (jq + 1) * 512)
                            xo = xop.tile([128, DC, 512], FP32, tag="xo",
                                          bufs=2)
                            for dc in range(DC):
                                nc.sync.dma_start(xo[:, dc, :],
                                                  xow_r[:, dc, qsl])
                            nbb = (2 * jq + 2) * 2   # key blocks per phase
                            # (phase, pair) list; pair covers blocks 2u,2u+1
                            pairs = [(ph, u) for ph in range(2)
                                     for u in range(nbb // 2)]
                            pts = {}
                            dn = dnp.tile([1, 512], FP32, tag="dn")
                            for pi, (ph, u) in enumerate(pairs):
                                pt = ptp.tile([128, 2, 512], F8, tag="pt")
                                pts[(ph, u)] = pt
                                for half in range(2):
                                    bb = 2 * u + half
                                    kb = ph * (S // 256) + bb  # 128-block
                                    s_ps = sps.tile([128, 512], FP32,
                                                    tag="s")
                                    for i in range(DCP):
                                        nc.tensor.matmul(
                                            s_ps,
                                            KT[:, 2 * i:2 * i + 2,
                                               kb * 128:(kb + 1) * 128],
                                            QT[:, 2 * i:2 * i + 2, qsl],
                                            start=(i == 0),
                                            stop=(i == DCP - 1),
                                            perf_mode=DRM)
                                    br = bb - 4 * jq
                                    if br >= 0:   # diagonal-zone: mask
                                        pe = work.tile([128, 512], BF16,
                                                       tag="pe")
                                        nc.scalar.activation(
                                            pe, s_ps, ACT.Exp, scale=scale_q)
                                        nc.gpsimd.tensor_mul(
                                            pt[:, half, :], pe,
                                            maskt[:, ph * 4 + br, :])
                                    else:
                                        nc.scalar.activation(
                                            pt[:, half, :], s_ps, ACT.Exp,
                                            scale=scale_q)
                                nc.tensor.matmul(dn, ones2, pt,
                                                 start=(pi == 0),
                                                 stop=(pi == len(pairs) - 1),
                                                 perf_mode=DRM)
                            den = work.tile([1, 512], F32R, tag="den")
                            nc.vector.tensor_copy(den, dn)
                            with nc.allow_low_precision(
                                    reason="softmax denom"):
                                nc.vector.reciprocal(den, den)
                            den_sb = bcast(den, 512, bc1, tag="bcd")
                            ctxn = attw.tile([128, DC, 512], BF16,
                                             tag="ctxn", bufs=1)
                            for dc in range(DC):
                                cps = avp.tile([128, 512], FP32, tag="av")
                                for pi, (ph, u) in enumerate(pairs):
                                    kb = ph * 8 + 2 * u
                                    nc.tensor.matmul(
                                        cps,
                                        VT[:, kb:kb + 2,
                                           dc * 128:(dc + 1) * 128],
                                        pts[(ph, u)],
                                        start=(pi == 0),
                                        stop=(pi == len(pairs) - 1),
                                        perf_mode=DRM)
                                nc.vector.tensor_mul(ctxn[:, dc, :], cps,
                                                     den_sb[:, 0, :])
                            # O projection (bf16) + residual
                            for ec in range(DC):
                                ops_t = avp.tile([128, 512], FP32, tag="av")
                                for dc in range(DC):
                                    nc.tensor.matmul(
                                        ops_t,
                                        WO[:, dc, ec * 128:(ec + 1) * 128],
                                        ctxn[:, dc, :],
                                        start=(dc == 0), stop=(dc == DC - 1))
                                nc.vector.tensor_add(
                                    y1[:, ec, qsl], ops_t, xo[:, ec, :])

                # ---- LN2 + FFN (shared scope: L1 closed above frees
                # KT/VT/QT, so fc1/w2-DMA overlap the LN2 window) ----
                with (
                    tc.tile_pool(name="y8p", bufs=1) as y8p,
                    tc.tile_pool(name="sq2p", bufs=2) as sq2p,
                    tc.tile_pool(name="ln2ps", bufs=1,
                                 space="PSUM") as ln2ps,
                    tc.tile_pool(name="bc2", bufs=2, space="PSUM") as bc2,
                    tc.tile_pool(name="hpool", bufs=1) as hp,
                    tc.tile_pool(name="w1p", bufs=2) as w1p,
                    tc.tile_pool(name="w2p", bufs=1) as w2p,
                    tc.tile_pool(name="fps", bufs=4, space="PSUM") as fps,
                    tc.tile_pool(name="otp", bufs=3) as otp,
                ):
                    h = hp.tile([128, FC, OWN], F8, tag="h")
                    w2 = w2p.tile([128, FC, D], F8, tag="w2")
                    for fc in range(FC):
                        nc.sync.dma_start(w2[:, fc, :], f28_r[:, fc, :])
                    y8 = y8p.tile([128, DC, OWN], F8, tag="y8")
                    for dc in range(DC):
                        nc.scalar.activation(y8[:, dc, :], y1[:, dc, :],
                                             ACT.Copy)
                    for tj in range(NTO):
                        sl = slice(tj * 512, (tj + 1) * 512)
                        ps_sx, ps_sq = ln_stats(y8, sl, ln2ps, sq2p,
                                                True)
                        mu, rs = ln_rows(ps_sx, ps_sq, 512)
                        mb_sb = bcast(mu, 512, bc2)
                        rb_sb = bcast(rs, 512, bc2)
                        ln_apply(y1, sl, xn2, sl, mb_sb, rb_sb, 2, 3)
                    if True:
                        # per token tile: fc1 then fc2 -> fc2(tj) overlaps
                        # fc1(tj+1) on the PE stream
                        for tj in range(NTO):
                            sl = slice(tj * 512, (tj + 1) * 512)
                            for quar in range(4):
                                w1 = w1p.tile([128, DC, F // 4], F8,
                                              tag="w1")
                                for dc in range(DC):
                                    nc.sync.dma_start(
                                        w1[:, dc, :],
                                        f18_r[:, dc,
                                              quar * (F // 4):
                                              (quar + 1) * (F // 4)])
                                for fi in range(DC):
                                    fc = quar * DC + fi
                                    ps = fps.tile([128, 512], FP32,
                                                  tag="fps")
                                    for i in range(DCP):
                                        nc.tensor.matmul(
                                            ps,
                                            w1[:, 2 * i:2 * i + 2,
                                               fi * 128:(fi + 1) * 128],
                                            xn2[:, 2 * i:2 * i + 2, sl],
                                            start=(i == 0),
                                            stop=(i == DCP - 1),
                                            perf_mode=DRM)
                                    nc.scalar.activation(
                                        h[:, fc, sl], ps, ACT.Relu,
                                        bias=fc1b_t[:, fc:fc + 1],
                                        scale=1.0 / WS)
                            for dc in range(DC):
                                ps = fps.tile([128, 512], FP32, tag="fps")
                                for i in range(FCP):
                                    nc.tensor.matmul(
                                        ps,
                                        w2[:, 2 * i:2 * i + 2,
                                           dc * 128:(dc + 1) * 128],
                                        h[:, 2 * i:2 * i + 2, sl],
                                        start=(i == 0), stop=(i == FCP - 1),
                                        perf_mode=DRM)
                                ot = otp.tile([128, 512], FP32, tag="ot")
                                nc.scalar.activation(
                                    ot, ps, ACT.Identity,
                                    bias=fc2b_t[:, dc:dc + 1],
                                    scale=1.0 / WS2)
                                nc.gpsimd.tensor_add(ot, ot, y1[:, dc, sl])
                                nc.sync.dma_start(outT_r[:, dc, sl], ot)

    if surgery:
        _split_drain_waits(nc)
    return nc


# ---------------- host side ----------------

_NC_CACHE = {}


def _get_nc(S, D, F, apply_ln_affine, repeat=1):
    key = (S, D, F, apply_ln_affine, repeat)
    if key not in _NC_CACHE:
        _NC_CACHE[key] = build_decoder_nc(S, D, F, apply_ln_affine,
                                          repeat=repeat)
    return _NC_CACHE[key]


def make_in_maps(x, W_q, W_k, W_v, W_o, fc1_w, fc1_b, fc2_w, fc2_b,
                 ln1_g, ln1_b, ln2_g, ln2_b, apply_ln_affine):
    B, S, D = x.shape
    f8 = ml_dtypes.float8_e4m3
    shared = {
        "wq8": np.ascontiguousarray(W_q.T * WS).astype(f8),
        "wk8": np.ascontiguousarray(W_k.T * WS).astype(f8),
        "wv8": np.ascontiguousarray(W_v.T * WS).astype(f8),
        "wob": np.ascontiguousarray(W_o.T).astype(ml_dtypes.bfloat16),
        "f18": np.ascontiguousarray(fc1_w.T * WS).astype(f8),
        "f28": np.ascontiguousarray(fc2_w.T * WS2).astype(f8),
        "fc1b": np.ascontiguousarray(fc1_b, dtype=np.float32),
        "fc2b": np.ascontiguousarray(fc2_b, dtype=np.float32),
    }
    if apply_ln_affine:
        shared["lnp"] = np.ascontiguousarray(
            np.stack([ln1_g, ln1_b, ln2_g, ln2_b]), dtype=np.float32)
    nj = S // 512
    in_maps, stripes = [], []
    for c in range(2 * B):
        b, hh = c // 2, c % 2
        # permuted token order: own phase first
        perm = np.concatenate(
            [j * 512 + ph * 256 + np.arange(256)
             for ph in (hh, 1 - hh) for j in range(nj)])
        stripe = (np.arange(S) % 512) // 256 == hh
        stripes.append((b, stripe))
        xTb = np.ascontiguousarray(x[b].T, dtype=np.float32)
        xperm = xTb[:, perm]
        # mask: rows r = php*512 + br*128 + p; cols c over 512 queries
        php = np.arange(1024)[:, None] // 512
        br = (np.arange(1024)[:, None] // 128) % 4
        p = np.arange(1024)[:, None] % 128
        ph_act = np.where(php == 0, hh, 1 - hh)
        kg = (br // 2) * 512 + ph_act * 256 + (br % 2) * 128 + p
        cq = np.arange(512)[None, :]
        qg = (cq // 256) * 512 + hh * 256 + (cq % 256)
        m = (kg <= qg).astype(ml_dtypes.bfloat16)
        in_maps.append(dict(shared,
                            xbf=np.ascontiguousarray(xperm).astype(
                                ml_dtypes.bfloat16),
                            xow=np.ascontiguousarray(xperm[:, :S // 2]),
                            mask8=m))
    return in_maps, stripes


def run_decoder(x, W_q, W_k, W_v, W_o, fc1_w, fc1_b, fc2_w, fc2_b,
                ln1_g, ln1_b, ln2_g, ln2_b, trace=False):
    x = np.asarray(x, dtype=np.float32)
    B, S, D = x.shape
    F = fc1_w.shape[0]
    apply_ln_affine = not (
        np.all(np.asarray(ln1_g) == 1.0) and np.all(np.asarray(ln1_b) == 0.0)
        and np.all(np.asarray(ln2_g) == 1.0)
        and np.all(np.asarray(ln2_b) == 0.0))
    nc = _get_nc(S, D, F, apply_ln_affine)
    in_maps, stripes = make_in_maps(
        x, np.asarray(W_q), np.asarray(W_k), np.asarray(W_v),
        np.asarray(W_o), np.asarray(fc1_w), np.asarray(fc1_b),
        np.asarray(fc2_w), np.asarray(fc2_b), np.asarray(ln1_g),
        np.asarray(ln1_b), np.asarray(ln2_g), np.asarray(ln2_b),
        apply_ln_affine)
    res = run_bass_kernel_spmd(nc, in_maps, core_ids=list(range(2 * B)),
                               trace=trace)
    out = np.empty((B, S, D), dtype=np.float32)
    for c in range(2 * B):
        b, stripe = stripes[c]
        out[b, stripe, :] = res.results[c]["outT"].T
    return out, res


def kernel(**inputs):
    out, _ = run_decoder(**inputs)
    return out


def _build_pjrt_fn(nc, in_maps):
    """Build a non-donating jitted executor + device-resident args."""
    import jax
    from jax.sharding import Mesh, PartitionSpec, NamedSharding
    from jax.experimental.shard_map import shard_map
    from concourse import bass2jax

    n_cores = len(in_maps)
    bass2jax.install_neuronx_cc_hook()
    partition_name = (nc.partition_id_tensor.name
                      if nc.partition_id_tensor else None)
    in_names, out_names, out_avals, zero_outs = [], [], [], []
    for alloc in nc.m.functions[0].allocations:
        if not isinstance(alloc, mybir.MemoryLocationSet):
            continue
        name = alloc.memorylocations[0].name
        if alloc.kind == "ExternalInput":
            if name != partition_name:
                in_names.append(name)
        elif alloc.kind == "ExternalOutput":
            shape = tuple(alloc.tensor_shape)
            dtype = mybir.dt.np(alloc.dtype)
            out_names.append(name)
            out_avals.append(jax.core.ShapedArray(shape, dtype))
            zero_outs.append(np.zeros(shape, dtype))
    n_params = len(in_names)
    in_names.extend(out_names)
    if partition_name is not None:
        in_names.append(partition_name)

    def _body(*args):
        operands = list(args)
        if partition_name is not None:
            operands.append(bass2jax.partition_id_tensor())
        return tuple(bass2jax._bass_exec_p.bind(
            *operands, out_avals=tuple(out_avals), in_names=tuple(in_names),
            out_names=tuple(out_names), lowering_input_output_aliases=(),
            sim_require_finite=True, sim_require_nnan=True, nc=nc))

    devices = jax.devices()[:n_cores]
    mesh = Mesh(np.asarray(devices), ("core",))
    fn = jax.jit(shard_map(
        _body, mesh=mesh,
        in_specs=(PartitionSpec("core"),) * (n_params + len(out_names)),
        out_specs=(PartitionSpec("core"),) * len(out_names),
        check_rep=False), keep_unused=True)
    sh = NamedSharding(mesh, PartitionSpec("core"))
    args = []
    for i in range(n_params):
        cat = np.concatenate([np.asarray(in_maps[c][in_names[i]])
                              for c in range(n_cores)], axis=0)
        args.append(jax.device_put(cat, sh))
    for z in zero_outs:
        args.append(jax.device_put(
            np.zeros((n_cores * z.shape[0], *z.shape[1:]), z.dtype), sh))
    return fn, args


def measure_body_ns(trials=4, n1=16, n2=96, **inputs):
    """Isolate per-execution NEFF body time from dispatch overhead: slope of
    async-pipelined executions, differenced between repeat=1 and repeat=4
    NEFFs.  Interleaved trials; min-slope estimator (dispatch noise is
    one-sided).  Returns (body_ns, slope1_ns)."""
    import time
    import jax

    x = np.asarray(inputs["x"], dtype=np.float32)
    B, S, D = x.shape
    F = np.asarray(inputs["fc1_w"]).shape[0]
    in_maps, _ = make_in_maps(
        x, *[np.asarray(inputs[k]) for k in
             ("W_q", "W_k", "W_v", "W_o", "fc1_w", "fc1_b", "fc2_w", "fc2_b",
              "ln1_g", "ln1_b", "ln2_g", "ln2_b")], False)

    fns = {}
    for rep in (1, 8):
        nc = _get_nc(S, D, F, False, repeat=rep)
        fn, args = _build_pjrt_fn(nc, in_maps)
        jax.block_until_ready(fn(*args))
        fns[rep] = (fn, args)

    def slope(rep):
        fn, args = fns[rep]
        ts = {}
        for N in (n1, n2):
            t0 = time.perf_counter()
            for _i in range(N):
                o = fn(*args)
            jax.block_until_ready(o)
            ts[N] = time.perf_counter() - t0
        return (ts[n2] - ts[n1]) / (n2 - n1)

    diffs, s1s = [], []
    for _t in range(trials):
        s1 = slope(1)
        s8 = slope(8)
        s1s.append(s1)
        diffs.append((s8 - s1) / 7 * 1e9)
    diffs.sort()
    body = diffs[len(diffs) // 2]
    return body, min(s1s) * 1e9
